# revision 4
# baseline (speedup 1.0000x reference)
"""Trainium2 Bass kernel for a dense transformer block (pre-LN, 8-head causal
attention + FFN), data-parallel over batch across 8 NeuronCores.

v2: feature-major [c, t] layout end-to-end.

  * Host pre-transposes x (f16) to [C, TOK] and post-transposes the [C, TOK]
    f16 output -- no on-device transposes or DRAM scratch bounces at all.
  * LN stats over the channel (partition) dim via PE ones-matmuls into an
    [8, 128] psum block (rows 0-3 mean per t-tile, 4-7 E[x^2]); the rsqrt
    Newton chain then runs on [4, 128] shapes (128-wide lanes, not [1, t]
    rows).  mean/rstd broadcast back to [128, t] with K=1 ones-matmuls.
  * Attention output computed directly transposed: per head the matmul uses
    V as the stationary operand (lhsT [s, 64] zero-padded) and scores^T
    [s, t] as the moving one, yielding attn^T [d, t] at partition bases
    0/64 -- which feeds the projection without any layout change.  Softmax
    denominators ride along as ones-columns ([1a,va,z,1b,vb,z] -> psum rows
    0 and 64), are approx-reciprocal'd straight out of PSUM, and broadcast
    per-head with K=1 all-ones matmuls.
  * Projection and FFN-W2 run in transposed orientation (out [c', t]), so
    residuals accumulate in [c, t] as well.  All activations f16.
  * 3-deep software pipeline across 512-token groups (stage order per
    iteration: LN1-stats(g) | QKV(g-1) | LN1-post(g) | attn(g-1) |
    LN2-post+FFN(g-2) | proj+LN2-stats(g-1)) so every PE wait on a DVE
    dependency is preceded by a large matmul block from another group --
    keeps the PE queue busy and the HAM clock warm.
  * PSUM: 8 banks split as main pool (4) + score tiles (2) + stats (2).
"""

import numpy as np

import concourse.bass as bass
import concourse.mybir as mybir
import concourse.tile as tile
from concourse import bacc
from concourse.bass_utils import run_bass_kernel_spmd

F32 = mybir.dt.float32
F16 = mybir.dt.float16
I32 = mybir.dt.int32
I16 = mybir.dt.int16

# Model dims
B, T, C = 128, 256, 384
H, HS = 8, 48
FF = 4 * C           # 1536
EPS = 1e-5

# Sharding / tiling
NCORES = 8
NB = B // NCORES     # 16 sequences per core
TOK = NB * T         # 4096 tokens per core
P = 128
CCH = C // P         # 3 c-chunks
FCH = FF // P        # 12 ffn chunks
DPAD = 512           # q/k head-padded dim (4 tiles x 2 heads x 64)
QMT = DPAD // P      # 4
NPAIR = H // 2       # 4 head pairs
PW = 128             # [1a, va(48), z(15), 1b, vb(48), z(15)] per pair
VW = NPAIR * PW      # 512 augmented v width
GT = 512             # tokens per group (2 sequences)
NG = TOK // GT       # 8 groups
GTT = GT // P        # 4 token tiles per group
ISCALE = float(HS) ** -0.5
MAGIC = 0x5F3759DF
MAGIC16 = 0x59BA
RC = 1.0 / C


def _build_program(flags):
    nc = bacc.Bacc(None, target_bir_lowering=False, debug=False)

    xt_d = nc.dram_tensor("xt", [C, TOK], F16, kind="ExternalInput").ap()
    wq_d = nc.dram_tensor("wq", [CCH, P, DPAD], F16, kind="ExternalInput").ap()
    wk_d = nc.dram_tensor("wk", [CCH, P, DPAD], F16, kind="ExternalInput").ap()
    wv_d = nc.dram_tensor("wv", [CCH, P, C], F16, kind="ExternalInput").ap()
    wp_d = nc.dram_tensor("wp", [QMT, P, C], F16, kind="ExternalInput").ap()
    w1_d = nc.dram_tensor("w1", [CCH, P, FF], F16, kind="ExternalInput").ap()
    w2_d = nc.dram_tensor("w2", [FCH, P, C], F16, kind="ExternalInput").ap()
    rowq_d = nc.dram_tensor("rowq", [1, DPAD], F16, kind="ExternalInput").ap()
    rowk_d = nc.dram_tensor("rowk", [1, DPAD], F16, kind="ExternalInput").ap()
    rowv_d = nc.dram_tensor("rowv", [1, C], F16, kind="ExternalInput").ap()
    rowp_d = nc.dram_tensor("rowp", [1, C], F16, kind="ExternalInput").ap()
    rowl_d = nc.dram_tensor("rowl", [1, C], F16, kind="ExternalInput").ap()
    b1t_d = nc.dram_tensor("b1t", [P, FCH], F32, kind="ExternalInput").ap()
    mask_d = nc.dram_tensor("maskmul", [P, 2 * P], F16, kind="ExternalInput").ap()
    out_d = nc.dram_tensor("out", [C, TOK], F16, kind="ExternalOutput").ap()

    with tile.TileContext(nc) as tc:
        with nc.allow_low_precision(reason="f16 activations within tolerance"):
            _emit(nc, tc, flags, xt_d, wq_d, wk_d, wv_d, wp_d, w1_d, w2_d,
                  rowq_d, rowk_d, rowv_d, rowp_d, rowl_d, b1t_d, mask_d,
                  out_d)
    nc.compile()
    return nc


def _emit(nc, tc, flags, xt_d, wq_d, wk_d, wv_d, wp_d, w1_d, w2_d,
          rowq_d, rowk_d, rowv_d, rowp_d, rowl_d, b1t_d, mask_d,
          out_d):
    from contextlib import ExitStack
    with ExitStack() as ctx:
        const = ctx.enter_context(tc.tile_pool(name="const", bufs=1))
        ln = ctx.enter_context(tc.tile_pool(name="ln", bufs=2))
        grp = ctx.enter_context(tc.tile_pool(name="grp", bufs=2))
        att = ctx.enter_context(tc.tile_pool(name="att", bufs=4))
        outp = ctx.enter_context(tc.tile_pool(name="outp", bufs=2))
        psum = ctx.enter_context(tc.tile_pool(name="psum", bufs=4, space="PSUM"))
        spp = ctx.enter_context(tc.tile_pool(name="spp", bufs=2, space="PSUM"))
        stp = ctx.enter_context(tc.tile_pool(name="stp", bufs=2, space="PSUM"))

        def ps_tile():
            return psum.tile([P, GT], F32, name="ps", tag="ps")

        # ---- constants ----
        wq_sb = const.tile([P, CCH, DPAD], F16)
        wk_sb = const.tile([P, CCH, DPAD], F16)
        wv_sb = const.tile([P, CCH, C], F16)
        wp_sb = const.tile([P, QMT, C], F16)
        w1_sb = const.tile([P, CCH, FF], F16)
        w2_sb = const.tile([P, FCH, C], F16)
        for cc in range(CCH):
            nc.sync.dma_start(wq_sb[:, cc, :], wq_d[cc])
            nc.sync.dma_start(wk_sb[:, cc, :], wk_d[cc])
            nc.sync.dma_start(wv_sb[:, cc, :], wv_d[cc])
            nc.sync.dma_start(w1_sb[:, cc, :], w1_d[cc])
        for m in range(QMT):
            nc.sync.dma_start(wp_sb[:, m, :], wp_d[m])
        for fc in range(FCH):
            nc.sync.dma_start(w2_sb[:, fc, :], w2_d[fc])
        mask_sb = const.tile([P, 2 * P], F16)
        nc.sync.dma_start(mask_sb, mask_d)
        mask3_sb = mask_sb.rearrange("p (b c) -> p b c", c=P)

        ones_sb = const.tile([1, GT], F16)
        nc.vector.memset(ones_sb, 1.0)
        onesc_sb = const.tile([P, 1], F16)   # 1/C column (stats stationary)
        nc.vector.memset(onesc_sb, RC)
        # all-ones [128,128]: K=1 stationary rows at any legal base partition
        ob_sb = const.tile([P, P], F16)
        nc.vector.memset(ob_sb, 1.0)

        rowq_sb = const.tile([1, DPAD], F16)
        rowk_sb = const.tile([1, DPAD], F16)
        rowv_sb = const.tile([1, C], F16)
        rowp_sb = const.tile([1, C], F16)
        rowl_sb = const.tile([1, C], F16)
        b1t_sb = const.tile([P, FCH], F32)
        if flags["rowq"]:
            nc.sync.dma_start(rowq_sb, rowq_d)
        if flags["rowk"]:
            nc.sync.dma_start(rowk_sb, rowk_d)
        if flags["rowv"]:
            nc.sync.dma_start(rowv_sb, rowv_d)
        if flags["rowp"]:
            nc.sync.dma_start(rowp_sb, rowp_d)
        if flags["rowl"]:
            nc.sync.dma_start(rowl_sb, rowl_d)
        if flags["b1t"]:
            nc.sync.dma_start(b1t_sb, b1t_d)

        # Per-group state carried across pipeline stages
        st = [dict() for _ in range(NG)]

        def stats_mms(ps_st, srcs):
            """6 stats matmuls: psum row 0 = mean row [1, GT], psum row 32 =
            E[x^2] row.  srcs = [(x_chunk, xsq_chunk)] * CCH."""
            for cc in range(CCH):
                xc, xq = srcs[cc]
                nc.tensor.matmul(ps_st[0:1, :], lhsT=onesc_sb, rhs=xc,
                                 start=(cc == 0), stop=(cc == CCH - 1))
                nc.tensor.matmul(ps_st[32:33, :], lhsT=onesc_sb, rhs=xq,
                                 start=(cc == 0), stop=(cc == CCH - 1))

        def rows_math(ps_st, tag):
            """rstd row via 32x32 block-transpose trick: the block-local
            transpose permutation is self-inverse, so math runs on 128-wide
            lanes and the result lands back as a [1, GT] row at partition 0.
            Returns (sA, rr): mean row = sA[0:1, 0, :], rstd row = rr[0:1,:].
            """
            sA = ln.tile([32, 2, GT], F32, tag=f"sA{tag}")
            F = ln.tile([32, 2, GT], F32, tag=f"F{tag}")
            W2 = ln.tile([32, 2, GT], F16, tag=f"W2{tag}")
            rr2 = ln.tile([32, 2, GT], F16, tag=f"rr2{tag}")
            nc.vector.tensor_copy(sA[0:1, 0, :], ps_st[0:1, :])
            nc.vector.tensor_copy(sA[0:1, 1, :], ps_st[32:33, :])
            nc.vector.transpose(F, sA)
            vm = F[:, 0, :].rearrange("p (a b) -> p a b", b=32)[:, :, 0]
            ve = F[:, 1, :].rearrange("p (a b) -> p a b", b=32)[:, :, 0]
            vwr = W2[:, 0, :].rearrange("p (a b) -> p a b", b=32)[:, :, 0]
            vwm = W2[:, 1, :].rearrange("p (a b) -> p a b", b=32)[:, :, 0]
            t_s = ln.tile([32, GT // 32], F32, tag=f"t{tag}")
            v_s = ln.tile([32, GT // 32], F32, tag=f"v{tag}")
            y_s = ln.tile([32, GT // 32], F32, tag=f"y{tag}")
            nc.vector.tensor_mul(t_s, vm, vm)
            # var = E[x^2] - mean^2 (eps negligible here, skipped)
            nc.vector.tensor_sub(v_s, ve, t_s)
            vi = v_s.bitcast(I32)
            yi = y_s.bitcast(I32)
            nc.vector.tensor_scalar(out=yi, in0=vi, scalar1=1, scalar2=0,
                                    op0=mybir.AluOpType.arith_shift_right,
                                    op1=mybir.AluOpType.arith_shift_right)
            nc.vector.tensor_scalar(out=yi, in0=yi, scalar1=-1,
                                    scalar2=MAGIC,
                                    op0=mybir.AluOpType.mult,
                                    op1=mybir.AluOpType.add)
            for _ in range(flags["newton"]):
                nc.vector.tensor_mul(t_s, y_s, y_s)
                nc.vector.tensor_mul(t_s, t_s, v_s)
                nc.vector.tensor_scalar(out=t_s, in0=t_s, scalar1=-0.5,
                                        scalar2=1.5,
                                        op0=mybir.AluOpType.mult,
                                        op1=mybir.AluOpType.add)
                nc.vector.tensor_mul(y_s, y_s, t_s)
            nc.vector.tensor_copy(vwr, y_s)
            nc.vector.tensor_copy(vwm, vm)
            nc.vector.transpose(rr2, W2)
            return rr2

        def bcast_mms(rr2):
            """K=1 ones-matmuls -> mean_b, rstd_b [128, GT] psum tiles."""
            mean_b = ps_tile()
            rstd_b = ps_tile()
            nc.tensor.matmul(mean_b, lhsT=ob_sb[0:1, :], rhs=rr2[0:1, 1, :],
                             start=True, stop=True)
            nc.tensor.matmul(rstd_b, lhsT=ob_sb[0:1, :], rhs=rr2[0:1, 0, :],
                             start=True, stop=True)
            return mean_b, rstd_b

        def ln_apply(src3, mean_b, rstd_b, dst3):
            for cc in range(CCH):
                t1 = ln.tile([P, GT], F16, tag="t1")
                nc.vector.tensor_sub(t1, src3[:, cc, :], mean_b)
                nc.vector.tensor_mul(dst3[:, cc, :], t1, rstd_b)

        # ============ stage A: load x + LN1 stats for group g ============
        def emit_ln1_stats(g):
            s = st[g]
            xT = grp.tile([P, CCH, GT], F16, tag="xT", name="xT")
            s["xT"] = xT
            for cc in range(CCH):
                nc.sync.dma_start(xT[:, cc, :],
                                  xt_d[cc * P:(cc + 1) * P,
                                       g * GT:(g + 1) * GT])
            ps_st = stp.tile([P, GT], F32, tag="st")
            s["st1"] = ps_st
            srcs = []
            for cc in range(CCH):
                xsq = ln.tile([P, GT], F16, tag="xsq")
                nc.vector.tensor_mul(xsq, xT[:, cc, :], xT[:, cc, :])
                srcs.append((xT[:, cc, :], xsq))
            stats_mms(ps_st, srcs)

        # ============ stage C: LN1 finish -> hT for group g ============
        def emit_ln1_post(g):
            s = st[g]
            rr2 = rows_math(s["st1"], "1")
            mean_b, rstd_b = bcast_mms(rr2)
            hT = grp.tile([P, CCH, GT], F16, tag="hT", name="hT")
            ln_apply(s["xT"], mean_b, rstd_b, hT)
            s["hT"] = hT

        # ============ stage B: QKV for group g ============
        def emit_qkv(g):
            s = st[g]
            hT = s["hT"]
            qT = grp.tile([P, QMT, GT], F16, tag="qT")
            kT = grp.tile([P, QMT, GT], F16, tag="kT")
            vaug = grp.tile([P, GTT, VW], F16, tag="vaug")
            s["qT"], s["kT"], s["vaug"] = qT, kT, vaug
            for dst, w_sb, row_sb, rowf in ((qT, wq_sb, rowq_sb, flags["rowq"]),
                                            (kT, wk_sb, rowk_sb, flags["rowk"])):
                for m in range(QMT):
                    ps = ps_tile()
                    for cc in range(CCH):
                        nc.tensor.matmul(ps, lhsT=w_sb[:, cc, m * P:(m + 1) * P],
                                         rhs=hT[:, cc, :],
                                         start=(cc == 0),
                                         stop=(cc == CCH - 1 and not rowf))
                    if rowf:
                        nc.tensor.matmul(ps, lhsT=row_sb[:, m * P:(m + 1) * P],
                                         rhs=ones_sb, start=False, stop=True)
                    nc.scalar.copy(dst[:, m, :], ps)
            # V natural [t, d]; interleaved pair layout [va, 1a, 1b, vb]
            for stt in range(GTT):
                ps = ps_tile()
                for cc in range(CCH):
                    nc.tensor.matmul(ps[:, :C],
                                     lhsT=hT[:, cc, stt * P:(stt + 1) * P],
                                     rhs=wv_sb[:, cc, :],
                                     start=(cc == 0),
                                     stop=(cc == CCH - 1 and not flags["rowv"]))
                if flags["rowv"]:
                    nc.tensor.matmul(ps[:, :C], lhsT=ones_sb[:, :P], rhs=rowv_sb,
                                     start=False, stop=True)
                v4 = vaug[:, stt, :].rearrange("p (q w) -> p q w", w=PW)
                p8 = ps[:, :C].rearrange("p (h w) -> p h w", w=HS)
                nc.scalar.copy(v4[:, :, 1:HS + 1], p8[:, 0::2, :])
                nc.scalar.copy(v4[:, :, 65:65 + HS], p8[:, 1::2, :])
                nc.vector.memset(v4[:, :, 0], 1.0)
                nc.vector.memset(v4[:, :, 64], 1.0)
                nc.vector.memset(v4[:, :, HS + 1:64], 0.0)
                nc.vector.memset(v4[:, :, 65 + HS:PW], 0.0)

        # ============ stage D: attention for group g ============
        # Emitted as chunk closures so FFN matmul chunks of group g-1 can
        # interleave between head pairs (keeps the in-order PE queue fed
        # while the softmax chain runs on DVE/ACT).
        def attn_chunks(g):
            s = st[g]
            qT, kT, vaug = s["qT"], s["kT"], s["vaug"]
            attnT = grp.tile([P, QMT, GT], F16, tag="attnT", name="attnT")
            s["attnT"] = attnT
            pair_ps = [None] * NPAIR
            rec2 = [None] * NPAIR

            def pair_mms(m):
                pair_ps[m] = ps_tile()
                ews = []
                for b2 in range(2):
                    s0 = b2 * T
                    for hh in range(2):
                        off = 64 * hh
                        sp = spp.tile([P, GT], F32, tag="sp", name="sp")
                        nc.tensor.matmul(
                            sp[:, :T],
                            lhsT=kT[off:off + HS, m, s0:s0 + P],
                            rhs=qT[off:off + HS, m, s0:s0 + T],
                            start=True, stop=False)
                        nc.tensor.matmul(
                            sp[:, T:T + P],
                            lhsT=kT[off:off + HS, m, s0 + P:s0 + T],
                            rhs=qT[off:off + HS, m, s0 + P:s0 + T],
                            start=False, stop=True)
                        ew = att.tile([P, T + P], F16, tag="ew", name="ew")
                        nc.scalar.activation(out=ew, in_=sp[:, :T + P],
                                             func=mybir.ActivationFunctionType.Exp,
                                             scale=ISCALE)
                        # causal mask on the two diagonal blocks only
                        ew3 = ew.rearrange("p (b c) -> p b c", c=P)[:, 0::2, :]
                        if hh == 0:
                            nc.vector.tensor_mul(ew3, ew3, mask3_sb)
                        else:
                            nc.gpsimd.tensor_mul(ew3, ew3, mask3_sb)
                        ews.append(ew)
                # attn^T += V^T scores^T ; v stationary, ew moving.
                # head-a -> rows 0:64 (den row 0); head-b -> rows 64:128
                # (den row 64); zero-padded v columns keep all rows written.
                pp = pair_ps[m]
                c0 = m * PW
                for b2 in range(2):
                    tb = b2 * T
                    for hh in range(2):
                        ew = ews[b2 * 2 + hh]
                        r0 = 64 * hh
                        ca = c0 + 64 * hh
                        nv = 64
                        nc.tensor.matmul(
                            pp[r0:r0 + nv, tb:tb + T],
                            lhsT=vaug[:, b2 * 2, ca:ca + nv],
                            rhs=ew[:, :T],
                            start=True, stop=False)
                        nc.tensor.matmul(
                            pp[r0:r0 + nv, tb + P:tb + T],
                            lhsT=vaug[:, b2 * 2 + 1, ca:ca + nv],
                            rhs=ew[:, T:T + P],
                            start=False, stop=True)
                rec32 = att.tile([65, GT], F32, tag="rec32", name="rec32")
                nc.vector.reciprocal_approx_fast(out=rec32,
                                                 in_=pair_ps[m][0:65, :])
                rec2[m] = att.tile([65, GT], F16, tag="rec2", name="rec2")
                nc.gpsimd.tensor_copy(rec2[m], rec32)

            def pair_fin(m):
                rec_ps = ps_tile()
                nc.tensor.matmul(rec_ps[0:64, :], lhsT=ob_sb[0:1, 0:64],
                                 rhs=rec2[m][0:1, :], start=True, stop=True)
                nc.tensor.matmul(rec_ps[64:P, :], lhsT=ob_sb[64:65, 0:64],
                                 rhs=rec2[m][64:65, :], start=True, stop=True)
                rec_b = att.tile([P, GT], F16, tag="recb", name="recb")
                nc.vector.tensor_copy(rec_b, rec_ps)
                nc.vector.tensor_mul(attnT[:, m, :], pair_ps[m], rec_b)

            def run_all():
                for m in range(NPAIR):
                    pair_mms(m)
                    if m > 0:
                        pair_fin(m - 1)
                pair_fin(NPAIR - 1)
            return run_all

        # ======= stage E: proj + residual + LN2 stats for group g =======
        def emit_proj_ln2stats(g):
            s = st[g]
            attnT = s["attnT"]
            x1T = grp.tile([P, CCH, GT], F16, tag="x1T")
            s["x1T"] = x1T
            ps_st = stp.tile([P, GT], F32, tag="st")
            s["st2"] = ps_st
            srcs = []
            for oc in range(CCH):
                ps = ps_tile()
                for m in range(QMT):
                    nc.tensor.matmul(ps, lhsT=wp_sb[:, m, oc * P:(oc + 1) * P],
                                     rhs=attnT[:, m, :],
                                     start=(m == 0),
                                     stop=(m == QMT - 1 and not flags["rowp"]))
                if flags["rowp"]:
                    nc.tensor.matmul(ps, lhsT=rowp_sb[:, oc * P:(oc + 1) * P],
                                     rhs=ones_sb, start=False, stop=True)
                nc.vector.tensor_add(x1T[:, oc, :], s["xT"][:, oc, :], ps)
                xsq = ln.tile([P, GT], F16, tag="xsq2")
                nc.vector.tensor_mul(xsq, x1T[:, oc, :], x1T[:, oc, :])
                srcs.append((x1T[:, oc, :], xsq))
            stats_mms(ps_st, srcs)

        # ============ stage E2: LN2 finish -> h2T for group g ============
        def emit_ln2_post(g):
            s = st[g]
            rr2 = rows_math(s["st2"], "2")
            mean_b, rstd_b = bcast_mms(rr2)
            h2T = grp.tile([P, CCH, GT], F16, tag="h2T", name="h2T")
            ln_apply(s["x1T"], mean_b, rstd_b, h2T)
            s["h2T"] = h2T

        # ============ stage F: LN2 finish + FFN + out for group g ============
        def emit_ffn(g):
            s = st[g]
            emit_ln2_post(g)
            h2T = s["h2T"]
            rg = grp.tile([P, FCH, GT], F16, tag="rg", name="rg")

            def w1_chunk(fcs):
                for fc in fcs:
                    ps = ps_tile()
                    for cc in range(CCH):
                        nc.tensor.matmul(ps,
                                         lhsT=w1_sb[:, cc, fc * P:(fc + 1) * P],
                                         rhs=h2T[:, cc, :],
                                         start=(cc == 0), stop=(cc == CCH - 1))
                    if flags["b1t"]:
                        if fc % 2 == 0:
                            nc.scalar.activation(
                                out=rg[:, fc, :], in_=ps,
                                func=mybir.ActivationFunctionType.Relu,
                                bias=b1t_sb[:, fc:fc + 1], scale=1.0)
                        else:
                            nc.vector.tensor_scalar(
                                out=rg[:, fc, :], in0=ps,
                                scalar1=b1t_sb[:, fc:fc + 1], scalar2=0.0,
                                op0=mybir.AluOpType.add,
                                op1=mybir.AluOpType.max)
                    else:
                        if fc % 2 == 0:
                            nc.scalar.activation(
                                out=rg[:, fc, :], in_=ps,
                                func=mybir.ActivationFunctionType.Relu,
                                scale=1.0)
                        else:
                            nc.vector.tensor_scalar_max(out=rg[:, fc, :],
                                                        in0=ps, scalar1=0.0)

            def w2_chunk(oc):
                ps = ps_tile()
                for fc in range(FCH):
                    nc.tensor.matmul(ps, lhsT=w2_sb[:, fc, oc * P:(oc + 1) * P],
                                     rhs=rg[:, fc, :],
                                     start=(fc == 0),
                                     stop=(fc == FCH - 1 and not flags["rowl"]))
                if flags["rowl"]:
                    nc.tensor.matmul(ps, lhsT=rowl_sb[:, oc * P:(oc + 1) * P],
                                     rhs=ones_sb, start=False, stop=True)
                ot = outp.tile([P, GT], F16, tag="ot", name="ot")
                nc.vector.tensor_add(ot, s["x1T"][:, oc, :], ps)
                nc.sync.dma_start(out_d[oc * P:(oc + 1) * P,
                                        g * GT:(g + 1) * GT], ot)

            w1_chunk(list(range(FCH)))
            for oc in range(CCH):
                w2_chunk(oc)
            s.clear()

        # ============ software pipeline ============
        # Iteration gi emits: ln1-stats(gi) | qkv(gi-1) | ln1-post(gi) |
        # attention(gi-1) | ln2-post+ffn(gi-2) | proj+ln2-stats(gi-1).
        for gi in range(NG + 2):
            if gi < NG:
                emit_ln1_stats(gi)
            if 1 <= gi <= NG:
                emit_qkv(gi - 1)
            if gi < NG:
                emit_ln1_post(gi)
            if 1 <= gi <= NG:
                attn_chunks(gi - 1)()
            if 2 <= gi:
                emit_ffn(gi - 2)
            if 1 <= gi <= NG:
                emit_proj_ln2stats(gi - 1)


def _prep_weights(Wq, Wk, Wv, Wproj, bproj, W1, b1, W2, b2, g1, beta1, g2,
                  beta2):
    f16 = np.float16
    g1 = g1.astype(np.float64)
    g2 = g2.astype(np.float64)

    def qk_pack(W):
        Ws = g1[None, :, None] * W.astype(np.float64)      # [H, C, HS]
        pad = np.zeros((CCH, P, DPAD), np.float64)
        row = np.zeros((1, DPAD), np.float64)
        beta_r = np.einsum('c,hcd->hd', beta1.astype(np.float64),
                           W.astype(np.float64))
        for h in range(H):
            m, hh = divmod(h, 2)
            col = m * P + 64 * hh
            pad[:, :, col:col + HS] = Ws[h].reshape(CCH, P, HS)
            row[0, col:col + HS] = beta_r[h]
        return pad.astype(f16), row.astype(f16)

    wq_pad, rowq = qk_pack(Wq)
    wk_pad, rowk = qk_pack(Wk)

    # V: concat-head layout [C, C]
    Wvs = (g1[None, :, None] * Wv.astype(np.float64))       # [H, C, HS]
    wv = np.transpose(Wvs, (1, 0, 2)).reshape(C, C)         # [c, h*HS+d]
    beta_v = np.einsum('c,hcd->hd', beta1.astype(np.float64),
                       Wv.astype(np.float64)).reshape(1, C)
    wv = wv.astype(f16).reshape(CCH, P, C)
    rowv = beta_v.astype(f16)

    # proj packed to the attn^T pair-row layout: chunk m holds heads
    # 2m (rows 1:49) and 2m+1 (rows 65:113); den/junk rows zero.
    Wp3 = Wproj.astype(np.float64).reshape(H, HS, C)
    wp = np.zeros((QMT, P, C), np.float64)
    for h in range(H):
        m, hh = divmod(h, 2)
        r0 = 1 if hh == 0 else 65
        wp[m, r0:r0 + HS, :] = Wp3[h]
    wp = wp.astype(f16)
    rowp = bproj.astype(f16).reshape(1, C)

    W1s = g2[:, None] * W1.astype(np.float64)
    w1p = W1s.astype(f16).reshape(CCH, P, FF)
    b1tot = (b1.astype(np.float64)
             + beta2.astype(np.float64) @ W1.astype(np.float64))
    b1t = b1tot.astype(np.float32).reshape(FCH, P).T.copy()   # [P, FCH]

    w2p = W2.astype(f16).reshape(FCH, P, C)
    rowl = b2.astype(f16).reshape(1, C)

    tri = np.triu(np.ones((P, P), np.float64))  # [s, t]: valid iff s <= t
    maskmul = np.concatenate([tri, tri], axis=1).astype(f16)

    wdict = dict(wq=wq_pad, wk=wk_pad, wv=wv, wp=wp, w1=w1p, w2=w2p,
                 rowq=rowq, rowk=rowk, rowv=rowv, rowp=rowp, rowl=rowl,
                 b1t=b1t, maskmul=maskmul)
    flags = {k: bool(np.any(wdict[k] != 0))
             for k in ("rowq", "rowk", "rowv", "rowp", "rowl", "b1t")}
    flags["newton"] = 0
    return wdict, flags


_CACHED = {}


def _get_program(flags):
    key = tuple(sorted(flags.items()))
    if key not in _CACHED:
        _CACHED[key] = _build_program(flags)
    return _CACHED[key]


def _run(inputs, trace=False):
    x = np.asarray(inputs["x"], np.float32)
    wdict, flags = _prep_weights(
        np.asarray(inputs["Wq"]), np.asarray(inputs["Wk"]),
        np.asarray(inputs["Wv"]), np.asarray(inputs["Wproj"]),
        np.asarray(inputs["bproj"]), np.asarray(inputs["W1"]),
        np.asarray(inputs["b1"]), np.asarray(inputs["W2"]),
        np.asarray(inputs["b2"]), np.asarray(inputs["g1"]),
        np.asarray(inputs["beta1"]), np.asarray(inputs["g2"]),
        np.asarray(inputs["beta2"]))

    shards = x.reshape(NCORES, TOK, C)
    in_maps = [dict(wdict,
                    xt=np.ascontiguousarray(
                        shards[i].T.astype(np.float16)))
               for i in range(NCORES)]
    nc = _get_program(flags)
    res = run_bass_kernel_spmd(nc, in_maps, list(range(NCORES)), trace=trace)
    out = np.stack([np.ascontiguousarray(res.results[i]["out"].T)
                    for i in range(NCORES)])
    return out.reshape(B, T, C).astype(np.float32), res


def kernel(**inputs):
    out, _ = _run(inputs, trace=False)
    return out


# revision 5
# speedup vs baseline: 1.0388x; 1.0388x over previous
"""Trainium2 Bass kernel for a dense transformer block (pre-LN, 8-head causal
attention + FFN), data-parallel over batch across 8 NeuronCores.

v2: feature-major [c, t] layout end-to-end.

  * Host pre-transposes x (f16) to [C, TOK] and post-transposes the [C, TOK]
    f16 output -- no on-device transposes or DRAM scratch bounces at all.
  * LN stats over the channel (partition) dim via PE ones-matmuls into an
    [8, 128] psum block (rows 0-3 mean per t-tile, 4-7 E[x^2]); the rsqrt
    Newton chain then runs on [4, 128] shapes (128-wide lanes, not [1, t]
    rows).  mean/rstd broadcast back to [128, t] with K=1 ones-matmuls.
  * Attention output computed directly transposed: per head the matmul uses
    V as the stationary operand (lhsT [s, 64] zero-padded) and scores^T
    [s, t] as the moving one, yielding attn^T [d, t] at partition bases
    0/64 -- which feeds the projection without any layout change.  Softmax
    denominators ride along as ones-columns ([1a,va,z,1b,vb,z] -> psum rows
    0 and 64), are approx-reciprocal'd straight out of PSUM, and broadcast
    per-head with K=1 all-ones matmuls.
  * Projection and FFN-W2 run in transposed orientation (out [c', t]), so
    residuals accumulate in [c, t] as well.  All activations f16.
  * 3-deep software pipeline across 512-token groups (stage order per
    iteration: LN1-stats(g) | QKV(g-1) | LN1-post(g) | attn(g-1) |
    LN2-post+FFN(g-2) | proj+LN2-stats(g-1)) so every PE wait on a DVE
    dependency is preceded by a large matmul block from another group --
    keeps the PE queue busy and the HAM clock warm.
  * PSUM: 8 banks split as main pool (4) + score tiles (2) + stats (2).
"""

import numpy as np

import concourse.bass as bass
import concourse.mybir as mybir
import concourse.tile as tile
from concourse import bacc
from concourse.bass_utils import run_bass_kernel_spmd

F32 = mybir.dt.float32
F16 = mybir.dt.float16
I32 = mybir.dt.int32
I16 = mybir.dt.int16

# Model dims
B, T, C = 128, 256, 384
H, HS = 8, 48
FF = 4 * C           # 1536
EPS = 1e-5

# Sharding / tiling
NCORES = 8
NB = B // NCORES     # 16 sequences per core
TOK = NB * T         # 4096 tokens per core
P = 128
CCH = C // P         # 3 c-chunks
FCH = FF // P        # 12 ffn chunks
DPAD = 512           # q/k head-padded dim (4 tiles x 2 heads x 64)
QMT = DPAD // P      # 4
NPAIR = H // 2       # 4 head pairs
PW = 128             # [1a, va(48), z(15), 1b, vb(48), z(15)] per pair
VW = NPAIR * PW      # 512 augmented v width
GT = 512             # tokens per group (2 sequences)
NG = TOK // GT       # 8 groups
GTT = GT // P        # 4 token tiles per group
ISCALE = float(HS) ** -0.5
MAGIC = 0x5F3759DF
MAGIC16 = 0x59BA
RC = 1.0 / C


def _build_program(flags):
    nc = bacc.Bacc(None, target_bir_lowering=False, debug=False)

    xt_d = nc.dram_tensor("xt", [C, TOK], F16, kind="ExternalInput").ap()
    wq_d = nc.dram_tensor("wq", [CCH, P, DPAD], F16, kind="ExternalInput").ap()
    wk_d = nc.dram_tensor("wk", [CCH, P, DPAD], F16, kind="ExternalInput").ap()
    wv_d = nc.dram_tensor("wv", [CCH, P, C], F16, kind="ExternalInput").ap()
    wp_d = nc.dram_tensor("wp", [QMT, P, C], F16, kind="ExternalInput").ap()
    w1_d = nc.dram_tensor("w1", [CCH, P, FF], F16, kind="ExternalInput").ap()
    w2_d = nc.dram_tensor("w2", [FCH, P, C], F16, kind="ExternalInput").ap()
    rowq_d = nc.dram_tensor("rowq", [1, DPAD], F16, kind="ExternalInput").ap()
    rowk_d = nc.dram_tensor("rowk", [1, DPAD], F16, kind="ExternalInput").ap()
    rowv_d = nc.dram_tensor("rowv", [1, C], F16, kind="ExternalInput").ap()
    rowp_d = nc.dram_tensor("rowp", [1, C], F16, kind="ExternalInput").ap()
    rowl_d = nc.dram_tensor("rowl", [1, C], F16, kind="ExternalInput").ap()
    b1t_d = nc.dram_tensor("b1t", [P, FCH], F32, kind="ExternalInput").ap()
    mask_d = nc.dram_tensor("maskmul", [P, 2 * P], F16, kind="ExternalInput").ap()
    out_d = nc.dram_tensor("out", [C, TOK], F16, kind="ExternalOutput").ap()

    with tile.TileContext(nc) as tc:
        with nc.allow_low_precision(reason="f16 activations within tolerance"):
            _emit(nc, tc, flags, xt_d, wq_d, wk_d, wv_d, wp_d, w1_d, w2_d,
                  rowq_d, rowk_d, rowv_d, rowp_d, rowl_d, b1t_d, mask_d,
                  out_d)
    nc.compile()
    return nc


def _emit(nc, tc, flags, xt_d, wq_d, wk_d, wv_d, wp_d, w1_d, w2_d,
          rowq_d, rowk_d, rowv_d, rowp_d, rowl_d, b1t_d, mask_d,
          out_d):
    from contextlib import ExitStack
    with ExitStack() as ctx:
        const = ctx.enter_context(tc.tile_pool(name="const", bufs=1))
        ln = ctx.enter_context(tc.tile_pool(name="ln", bufs=2))
        grp = ctx.enter_context(tc.tile_pool(name="grp", bufs=2))
        att = ctx.enter_context(tc.tile_pool(name="att", bufs=4))
        outp = ctx.enter_context(tc.tile_pool(name="outp", bufs=2))
        psum = ctx.enter_context(tc.tile_pool(name="psum", bufs=4, space="PSUM"))
        spp = ctx.enter_context(tc.tile_pool(name="spp", bufs=2, space="PSUM"))
        stp = ctx.enter_context(tc.tile_pool(name="stp", bufs=2, space="PSUM"))

        def ps_tile():
            return psum.tile([P, GT], F32, name="ps", tag="ps")

        # ---- constants ----
        wq_sb = const.tile([P, CCH, DPAD], F16)
        wk_sb = const.tile([P, CCH, DPAD], F16)
        wv_sb = const.tile([P, CCH, C], F16)
        wp_sb = const.tile([P, QMT, C], F16)
        w1_sb = const.tile([P, CCH, FF], F16)
        w2_sb = const.tile([P, FCH, C], F16)
        for cc in range(CCH):
            nc.sync.dma_start(wq_sb[:, cc, :], wq_d[cc])
            nc.sync.dma_start(wk_sb[:, cc, :], wk_d[cc])
            nc.sync.dma_start(wv_sb[:, cc, :], wv_d[cc])
            nc.sync.dma_start(w1_sb[:, cc, :], w1_d[cc])
        for m in range(QMT):
            nc.sync.dma_start(wp_sb[:, m, :], wp_d[m])
        for fc in range(FCH):
            nc.sync.dma_start(w2_sb[:, fc, :], w2_d[fc])
        mask_sb = const.tile([P, 2 * P], F16)
        nc.sync.dma_start(mask_sb, mask_d)
        mask3_sb = mask_sb.rearrange("p (b c) -> p b c", c=P)

        ones_sb = const.tile([1, GT], F16)
        nc.vector.memset(ones_sb, 1.0)
        onesc_sb = const.tile([P, 1], F16)   # 1/C column (stats stationary)
        nc.vector.memset(onesc_sb, RC)
        # all-ones [128,128]: K=1 stationary rows at any legal base partition
        ob_sb = const.tile([P, P], F16)
        nc.vector.memset(ob_sb, 1.0)

        rowq_sb = const.tile([1, DPAD], F16)
        rowk_sb = const.tile([1, DPAD], F16)
        rowv_sb = const.tile([1, C], F16)
        rowp_sb = const.tile([1, C], F16)
        rowl_sb = const.tile([1, C], F16)
        b1t_sb = const.tile([P, FCH], F32)
        if flags["rowq"]:
            nc.sync.dma_start(rowq_sb, rowq_d)
        if flags["rowk"]:
            nc.sync.dma_start(rowk_sb, rowk_d)
        if flags["rowv"]:
            nc.sync.dma_start(rowv_sb, rowv_d)
        if flags["rowp"]:
            nc.sync.dma_start(rowp_sb, rowp_d)
        if flags["rowl"]:
            nc.sync.dma_start(rowl_sb, rowl_d)
        if flags["b1t"]:
            nc.sync.dma_start(b1t_sb, b1t_d)

        # Per-group state carried across pipeline stages
        st = [dict() for _ in range(NG)]

        def stats_mms(ps_st, srcs):
            """6 stats matmuls: psum row 0 = mean row [1, GT], psum row 32 =
            E[x^2] row.  srcs = [(x_chunk, xsq_chunk)] * CCH."""
            for cc in range(CCH):
                xc, xq = srcs[cc]
                nc.tensor.matmul(ps_st[0:1, :], lhsT=onesc_sb, rhs=xc,
                                 start=(cc == 0), stop=(cc == CCH - 1))
                nc.tensor.matmul(ps_st[32:33, :], lhsT=onesc_sb, rhs=xq,
                                 start=(cc == 0), stop=(cc == CCH - 1))

        def rows_math(ps_st, tag):
            """rstd row via 32x32 block-transpose trick: the block-local
            transpose permutation is self-inverse, so math runs on 128-wide
            lanes and the result lands back as a [1, GT] row at partition 0.
            Returns (sA, rr): mean row = sA[0:1, 0, :], rstd row = rr[0:1,:].
            """
            sA = ln.tile([32, 2, GT], F32, tag=f"sA{tag}")
            F = ln.tile([32, 2, GT], F32, tag=f"F{tag}")
            W2 = ln.tile([32, 2, GT], F16, tag=f"W2{tag}")
            rr2 = ln.tile([32, 2, GT], F16, tag=f"rr2{tag}")
            nc.vector.tensor_copy(sA[0:1, 0, :], ps_st[0:1, :])
            nc.vector.tensor_copy(sA[0:1, 1, :], ps_st[32:33, :])
            nc.vector.transpose(F, sA)
            vm = F[:, 0, :].rearrange("p (a b) -> p a b", b=32)[:, :, 0]
            ve = F[:, 1, :].rearrange("p (a b) -> p a b", b=32)[:, :, 0]
            vwr = W2[:, 0, :].rearrange("p (a b) -> p a b", b=32)[:, :, 0]
            vwm = W2[:, 1, :].rearrange("p (a b) -> p a b", b=32)[:, :, 0]
            t_s = ln.tile([32, GT // 32], F32, tag=f"t{tag}")
            v_s = ln.tile([32, GT // 32], F32, tag=f"v{tag}")
            y_s = ln.tile([32, GT // 32], F32, tag=f"y{tag}")
            nc.vector.tensor_mul(t_s, vm, vm)
            # var = E[x^2] - mean^2 (eps negligible here, skipped)
            nc.vector.tensor_sub(v_s, ve, t_s)
            vi = v_s.bitcast(I32)
            yi = y_s.bitcast(I32)
            nc.vector.tensor_scalar(out=yi, in0=vi, scalar1=1, scalar2=0,
                                    op0=mybir.AluOpType.arith_shift_right,
                                    op1=mybir.AluOpType.arith_shift_right)
            nc.vector.tensor_scalar(out=yi, in0=yi, scalar1=-1,
                                    scalar2=MAGIC,
                                    op0=mybir.AluOpType.mult,
                                    op1=mybir.AluOpType.add)
            for _ in range(flags["newton"]):
                nc.vector.tensor_mul(t_s, y_s, y_s)
                nc.vector.tensor_mul(t_s, t_s, v_s)
                nc.vector.tensor_scalar(out=t_s, in0=t_s, scalar1=-0.5,
                                        scalar2=1.5,
                                        op0=mybir.AluOpType.mult,
                                        op1=mybir.AluOpType.add)
                nc.vector.tensor_mul(y_s, y_s, t_s)
            nc.vector.tensor_copy(vwr, y_s)
            nc.vector.tensor_copy(vwm, vm)
            nc.vector.transpose(rr2, W2)
            return rr2

        def bcast_mms(rr2):
            """K=1 ones-matmuls -> mean_b, rstd_b [128, GT] psum tiles."""
            mean_b = ps_tile()
            rstd_b = ps_tile()
            nc.tensor.matmul(mean_b, lhsT=ob_sb[0:1, :], rhs=rr2[0:1, 1, :],
                             start=True, stop=True)
            nc.tensor.matmul(rstd_b, lhsT=ob_sb[0:1, :], rhs=rr2[0:1, 0, :],
                             start=True, stop=True)
            return mean_b, rstd_b

        def ln_apply(src3, mean_b, rstd_b, dst3):
            for cc in range(CCH):
                t1 = ln.tile([P, GT], F16, tag="t1")
                nc.vector.tensor_sub(t1, src3[:, cc, :], mean_b)
                nc.vector.tensor_mul(dst3[:, cc, :], t1, rstd_b)

        # ============ stage A: load x + LN1 stats for group g ============
        def emit_ln1_stats(g):
            s = st[g]
            xT = grp.tile([P, CCH, GT], F16, tag="xT", name="xT")
            s["xT"] = xT
            for cc in range(CCH):
                nc.sync.dma_start(xT[:, cc, :],
                                  xt_d[cc * P:(cc + 1) * P,
                                       g * GT:(g + 1) * GT])
            ps_st = stp.tile([P, GT], F32, tag="st")
            s["st1"] = ps_st
            srcs = []
            for cc in range(CCH):
                xsq = ln.tile([P, GT], F16, tag="xsq")
                nc.vector.tensor_mul(xsq, xT[:, cc, :], xT[:, cc, :])
                srcs.append((xT[:, cc, :], xsq))
            stats_mms(ps_st, srcs)

        # ============ stage C: LN1 finish -> hT for group g ============
        def emit_ln1_post(g):
            s = st[g]
            rr2 = rows_math(s["st1"], "1")
            mean_b, rstd_b = bcast_mms(rr2)
            hT = grp.tile([P, CCH, GT], F16, tag="hT", name="hT")
            ln_apply(s["xT"], mean_b, rstd_b, hT)
            s["hT"] = hT

        # ============ stage B: QKV for group g ============
        def emit_qkv(g):
            s = st[g]
            hT = s["hT"]
            qT = grp.tile([P, QMT, GT], F16, tag="qT")
            kT = grp.tile([P, QMT, GT], F16, tag="kT")
            vaug = grp.tile([P, GTT, VW], F16, tag="vaug")
            s["qT"], s["kT"], s["vaug"] = qT, kT, vaug
            for dst, w_sb, row_sb, rowf in ((qT, wq_sb, rowq_sb, flags["rowq"]),
                                            (kT, wk_sb, rowk_sb, flags["rowk"])):
                for m in range(QMT):
                    ps = ps_tile()
                    for cc in range(CCH):
                        nc.tensor.matmul(ps, lhsT=w_sb[:, cc, m * P:(m + 1) * P],
                                         rhs=hT[:, cc, :],
                                         start=(cc == 0),
                                         stop=(cc == CCH - 1 and not rowf))
                    if rowf:
                        nc.tensor.matmul(ps, lhsT=row_sb[:, m * P:(m + 1) * P],
                                         rhs=ones_sb, start=False, stop=True)
                    nc.scalar.copy(dst[:, m, :], ps)
            # V natural [t, d]; interleaved pair layout [va, 1a, 1b, vb]
            for stt in range(GTT):
                ps = ps_tile()
                for cc in range(CCH):
                    nc.tensor.matmul(ps[:, :C],
                                     lhsT=hT[:, cc, stt * P:(stt + 1) * P],
                                     rhs=wv_sb[:, cc, :],
                                     start=(cc == 0),
                                     stop=(cc == CCH - 1 and not flags["rowv"]))
                if flags["rowv"]:
                    nc.tensor.matmul(ps[:, :C], lhsT=ones_sb[:, :P], rhs=rowv_sb,
                                     start=False, stop=True)
                v4 = vaug[:, stt, :].rearrange("p (q w) -> p q w", w=PW)
                p8 = ps[:, :C].rearrange("p (h w) -> p h w", w=HS)
                nc.scalar.copy(v4[:, :, 1:HS + 1], p8[:, 0::2, :])
                nc.scalar.copy(v4[:, :, 65:65 + HS], p8[:, 1::2, :])
                nc.vector.memset(v4[:, :, 0], 1.0)
                nc.vector.memset(v4[:, :, 64], 1.0)
                nc.vector.memset(v4[:, :, HS + 1:64], 0.0)
                nc.vector.memset(v4[:, :, 65 + HS:PW], 0.0)

        # ============ stage D: attention for group g ============
        # Emitted as chunk closures so FFN matmul chunks of group g-1 can
        # interleave between head pairs (keeps the in-order PE queue fed
        # while the softmax chain runs on DVE/ACT).
        def attn_chunks(g):
            s = st[g]
            qT, kT, vaug = s["qT"], s["kT"], s["vaug"]
            attnT = grp.tile([P, QMT, GT], F16, tag="attnT", name="attnT")
            s["attnT"] = attnT
            pair_ps = [None] * NPAIR
            rec2 = [None] * NPAIR

            def pair_mms(m):
                pair_ps[m] = ps_tile()
                ews = []
                for b2 in range(2):
                    s0 = b2 * T
                    for hh in range(2):
                        off = 64 * hh
                        sp = spp.tile([P, GT], F32, tag="sp", name="sp")
                        nc.tensor.matmul(
                            sp[:, :T],
                            lhsT=kT[off:off + HS, m, s0:s0 + P],
                            rhs=qT[off:off + HS, m, s0:s0 + T],
                            start=True, stop=False)
                        nc.tensor.matmul(
                            sp[:, T:T + P],
                            lhsT=kT[off:off + HS, m, s0 + P:s0 + T],
                            rhs=qT[off:off + HS, m, s0 + P:s0 + T],
                            start=False, stop=True)
                        ew = att.tile([P, T + P], F16, tag="ew", name="ew")
                        nc.scalar.activation(out=ew, in_=sp[:, :T + P],
                                             func=mybir.ActivationFunctionType.Exp,
                                             scale=ISCALE)
                        # causal mask on the two diagonal blocks only
                        ew3 = ew.rearrange("p (b c) -> p b c", c=P)[:, 0::2, :]
                        if hh == 0:
                            nc.vector.tensor_mul(ew3, ew3, mask3_sb)
                        else:
                            nc.gpsimd.tensor_mul(ew3, ew3, mask3_sb)
                        ews.append(ew)
                # attn^T += V^T scores^T ; v stationary, ew moving.
                # head-a -> rows 0:64 (den row 0); head-b -> rows 64:128
                # (den row 64); zero-padded v columns keep all rows written.
                pp = pair_ps[m]
                c0 = m * PW
                for b2 in range(2):
                    tb = b2 * T
                    for hh in range(2):
                        ew = ews[b2 * 2 + hh]
                        r0 = 64 * hh
                        ca = c0 + 64 * hh
                        nv = 64
                        nc.tensor.matmul(
                            pp[r0:r0 + nv, tb:tb + T],
                            lhsT=vaug[:, b2 * 2, ca:ca + nv],
                            rhs=ew[:, :T],
                            start=True, stop=False)
                        nc.tensor.matmul(
                            pp[r0:r0 + nv, tb + P:tb + T],
                            lhsT=vaug[:, b2 * 2 + 1, ca:ca + nv],
                            rhs=ew[:, T:T + P],
                            start=False, stop=True)
                rec32 = att.tile([65, GT], F32, tag="rec32", name="rec32")
                nc.vector.reciprocal_approx_fast(out=rec32,
                                                 in_=pair_ps[m][0:65, :])
                rec2[m] = att.tile([65, GT], F16, tag="rec2", name="rec2")
                nc.vector.tensor_copy(rec2[m], rec32)

            def pair_fin(m):
                rec_ps = ps_tile()
                nc.tensor.matmul(rec_ps[0:64, :], lhsT=ob_sb[0:1, 0:64],
                                 rhs=rec2[m][0:1, :], start=True, stop=True)
                nc.tensor.matmul(rec_ps[64:P, :], lhsT=ob_sb[64:65, 0:64],
                                 rhs=rec2[m][64:65, :], start=True, stop=True)
                rec_b = att.tile([P, GT], F16, tag="recb", name="recb")
                nc.vector.tensor_copy(rec_b, rec_ps)
                nc.vector.tensor_mul(attnT[:, m, :], pair_ps[m], rec_b)

            def run_all():
                for m in range(NPAIR):
                    pair_mms(m)
                    if m > 0:
                        pair_fin(m - 1)
                pair_fin(NPAIR - 1)
            return run_all

        # ======= stage E: proj + residual + LN2 stats for group g =======
        def emit_proj_ln2stats(g):
            s = st[g]
            attnT = s["attnT"]
            x1T = grp.tile([P, CCH, GT], F16, tag="x1T")
            s["x1T"] = x1T
            ps_st = stp.tile([P, GT], F32, tag="st")
            s["st2"] = ps_st
            srcs = []
            for oc in range(CCH):
                ps = ps_tile()
                for m in range(QMT):
                    nc.tensor.matmul(ps, lhsT=wp_sb[:, m, oc * P:(oc + 1) * P],
                                     rhs=attnT[:, m, :],
                                     start=(m == 0),
                                     stop=(m == QMT - 1 and not flags["rowp"]))
                if flags["rowp"]:
                    nc.tensor.matmul(ps, lhsT=rowp_sb[:, oc * P:(oc + 1) * P],
                                     rhs=ones_sb, start=False, stop=True)
                nc.vector.tensor_add(x1T[:, oc, :], s["xT"][:, oc, :], ps)
                xsq = ln.tile([P, GT], F16, tag="xsq2")
                nc.vector.tensor_mul(xsq, x1T[:, oc, :], x1T[:, oc, :])
                srcs.append((x1T[:, oc, :], xsq))
            stats_mms(ps_st, srcs)

        # ============ stage E2: LN2 finish -> h2T for group g ============
        def emit_ln2_post(g):
            s = st[g]
            rr2 = rows_math(s["st2"], "2")
            mean_b, rstd_b = bcast_mms(rr2)
            h2T = grp.tile([P, CCH, GT], F16, tag="h2T", name="h2T")
            ln_apply(s["x1T"], mean_b, rstd_b, h2T)
            s["h2T"] = h2T

        # ============ stage F: LN2 finish + FFN + out for group g ============
        def emit_ffn(g):
            s = st[g]
            emit_ln2_post(g)
            h2T = s["h2T"]
            rg = grp.tile([P, FCH, GT], F16, tag="rg", name="rg")

            def w1_chunk(fcs):
                for fc in fcs:
                    ps = ps_tile()
                    for cc in range(CCH):
                        nc.tensor.matmul(ps,
                                         lhsT=w1_sb[:, cc, fc * P:(fc + 1) * P],
                                         rhs=h2T[:, cc, :],
                                         start=(cc == 0), stop=(cc == CCH - 1))
                    if flags["b1t"]:
                        if fc % 2 == 0:
                            nc.scalar.activation(
                                out=rg[:, fc, :], in_=ps,
                                func=mybir.ActivationFunctionType.Relu,
                                bias=b1t_sb[:, fc:fc + 1], scale=1.0)
                        else:
                            nc.vector.tensor_scalar(
                                out=rg[:, fc, :], in0=ps,
                                scalar1=b1t_sb[:, fc:fc + 1], scalar2=0.0,
                                op0=mybir.AluOpType.add,
                                op1=mybir.AluOpType.max)
                    else:
                        if fc % 2 == 0:
                            nc.scalar.activation(
                                out=rg[:, fc, :], in_=ps,
                                func=mybir.ActivationFunctionType.Relu,
                                scale=1.0)
                        else:
                            nc.vector.tensor_scalar_max(out=rg[:, fc, :],
                                                        in0=ps, scalar1=0.0)

            def w2_chunk(oc):
                ps = ps_tile()
                for fc in range(FCH):
                    nc.tensor.matmul(ps, lhsT=w2_sb[:, fc, oc * P:(oc + 1) * P],
                                     rhs=rg[:, fc, :],
                                     start=(fc == 0),
                                     stop=(fc == FCH - 1 and not flags["rowl"]))
                if flags["rowl"]:
                    nc.tensor.matmul(ps, lhsT=rowl_sb[:, oc * P:(oc + 1) * P],
                                     rhs=ones_sb, start=False, stop=True)
                ot = outp.tile([P, GT], F16, tag="ot", name="ot")
                nc.vector.tensor_add(ot, s["x1T"][:, oc, :], ps)
                nc.sync.dma_start(out_d[oc * P:(oc + 1) * P,
                                        g * GT:(g + 1) * GT], ot)

            w1_chunk(list(range(FCH)))
            for oc in range(CCH):
                w2_chunk(oc)
            s.clear()

        # ============ software pipeline ============
        # Iteration gi emits: ln1-stats(gi) | qkv(gi-1) | ln1-post(gi) |
        # attention(gi-1) | ln2-post+ffn(gi-2) | proj+ln2-stats(gi-1).
        for gi in range(NG + 2):
            if gi < NG:
                emit_ln1_stats(gi)
            if 1 <= gi <= NG:
                emit_qkv(gi - 1)
            if gi < NG:
                emit_ln1_post(gi)
            if 1 <= gi <= NG:
                attn_chunks(gi - 1)()
            if 2 <= gi:
                emit_ffn(gi - 2)
            if 1 <= gi <= NG:
                emit_proj_ln2stats(gi - 1)


def _prep_weights(Wq, Wk, Wv, Wproj, bproj, W1, b1, W2, b2, g1, beta1, g2,
                  beta2):
    f16 = np.float16
    g1 = g1.astype(np.float64)
    g2 = g2.astype(np.float64)

    def qk_pack(W):
        Ws = g1[None, :, None] * W.astype(np.float64)      # [H, C, HS]
        pad = np.zeros((CCH, P, DPAD), np.float64)
        row = np.zeros((1, DPAD), np.float64)
        beta_r = np.einsum('c,hcd->hd', beta1.astype(np.float64),
                           W.astype(np.float64))
        for h in range(H):
            m, hh = divmod(h, 2)
            col = m * P + 64 * hh
            pad[:, :, col:col + HS] = Ws[h].reshape(CCH, P, HS)
            row[0, col:col + HS] = beta_r[h]
        return pad.astype(f16), row.astype(f16)

    wq_pad, rowq = qk_pack(Wq)
    wk_pad, rowk = qk_pack(Wk)

    # V: concat-head layout [C, C]
    Wvs = (g1[None, :, None] * Wv.astype(np.float64))       # [H, C, HS]
    wv = np.transpose(Wvs, (1, 0, 2)).reshape(C, C)         # [c, h*HS+d]
    beta_v = np.einsum('c,hcd->hd', beta1.astype(np.float64),
                       Wv.astype(np.float64)).reshape(1, C)
    wv = wv.astype(f16).reshape(CCH, P, C)
    rowv = beta_v.astype(f16)

    # proj packed to the attn^T pair-row layout: chunk m holds heads
    # 2m (rows 1:49) and 2m+1 (rows 65:113); den/junk rows zero.
    Wp3 = Wproj.astype(np.float64).reshape(H, HS, C)
    wp = np.zeros((QMT, P, C), np.float64)
    for h in range(H):
        m, hh = divmod(h, 2)
        r0 = 1 if hh == 0 else 65
        wp[m, r0:r0 + HS, :] = Wp3[h]
    wp = wp.astype(f16)
    rowp = bproj.astype(f16).reshape(1, C)

    W1s = g2[:, None] * W1.astype(np.float64)
    w1p = W1s.astype(f16).reshape(CCH, P, FF)
    b1tot = (b1.astype(np.float64)
             + beta2.astype(np.float64) @ W1.astype(np.float64))
    b1t = b1tot.astype(np.float32).reshape(FCH, P).T.copy()   # [P, FCH]

    w2p = W2.astype(f16).reshape(FCH, P, C)
    rowl = b2.astype(f16).reshape(1, C)

    tri = np.triu(np.ones((P, P), np.float64))  # [s, t]: valid iff s <= t
    maskmul = np.concatenate([tri, tri], axis=1).astype(f16)

    wdict = dict(wq=wq_pad, wk=wk_pad, wv=wv, wp=wp, w1=w1p, w2=w2p,
                 rowq=rowq, rowk=rowk, rowv=rowv, rowp=rowp, rowl=rowl,
                 b1t=b1t, maskmul=maskmul)
    flags = {k: bool(np.any(wdict[k] != 0))
             for k in ("rowq", "rowk", "rowv", "rowp", "rowl", "b1t")}
    flags["newton"] = 0
    return wdict, flags


_CACHED = {}


def _get_program(flags):
    key = tuple(sorted(flags.items()))
    if key not in _CACHED:
        _CACHED[key] = _build_program(flags)
    return _CACHED[key]


def _run(inputs, trace=False):
    x = np.asarray(inputs["x"], np.float32)
    wdict, flags = _prep_weights(
        np.asarray(inputs["Wq"]), np.asarray(inputs["Wk"]),
        np.asarray(inputs["Wv"]), np.asarray(inputs["Wproj"]),
        np.asarray(inputs["bproj"]), np.asarray(inputs["W1"]),
        np.asarray(inputs["b1"]), np.asarray(inputs["W2"]),
        np.asarray(inputs["b2"]), np.asarray(inputs["g1"]),
        np.asarray(inputs["beta1"]), np.asarray(inputs["g2"]),
        np.asarray(inputs["beta2"]))

    shards = x.reshape(NCORES, TOK, C)
    in_maps = [dict(wdict,
                    xt=np.ascontiguousarray(
                        shards[i].T.astype(np.float16)))
               for i in range(NCORES)]
    nc = _get_program(flags)
    res = run_bass_kernel_spmd(nc, in_maps, list(range(NCORES)), trace=trace)
    out = np.stack([np.ascontiguousarray(res.results[i]["out"].T)
                    for i in range(NCORES)])
    return out.reshape(B, T, C).astype(np.float32), res


def kernel(**inputs):
    out, _ = _run(inputs, trace=False)
    return out
